# revision 24
# baseline (speedup 1.0000x reference)
"""Trainium2 Bass kernel for a 2-layer SimpleRNN classifier (v3).

Model (per reference):
  x = emb[tokens]                               # [B,T,E]
  seq1 = SimpleRNN_relu(x;  W1x, W1h, b1)       # [B,T,H1], return_sequences
  h    = SimpleRNN_relu(seq1; W2x, W2h, b2)[-1] # [B,H2], last step
  h = relu(h@Wd1+bd1); h = relu(h@Wd2+bd2); out = sigmoid(h@Wc+bc)  # [B,1]

Sharding: data-parallel over batch, 8 rows per core on 8 NeuronCores.
Activations transposed on-chip (features on partitions, (time,batch) on
the free dim). Structure (v3, fp16 operands / fp32 PSUM):
  - the bulk input projections (xw) accumulate directly in PSUM
    (16-step blocks, one PSUM bank each, double buffered); the
    recurrent Wh matmuls accumulate on top (start=False) and the
    per-step relu is one fused tensor_scalar max from PSUM (no identity
    matmuls, no PSUM->SBUF xw copies);
  - b1 is folded into the embedding via a constant-one padding column
    (x_pad[...,300]=1, W1x_pad[300,:]=b1); b2 is preloaded into each
    PSUM block by ScalarE, emitted mid-block so it hides behind rnn1;
  - recurrent matmul order is k-interleaved so matmuls consuming the
    freshest state chunks issue as late as possible.
fp8 was measured and rejected: PE stationary load for fp8 has a
~107-132ns/instruction floor on this hw vs ~32ns for fp16, so one fp8
DoubleRow (2 chunks) loses to the two fp16 matmuls it replaces.
"""

import numpy as np

import concourse.bass as bass
import concourse.mybir as mybir
import concourse.tile as tile
from concourse.bass import _add_dep_helper
from concourse.bass_utils import run_bass_kernel_spmd

# ---------------------------------------------------------------------------
# Problem constants (hardcoded per the task contract).
B, T, V, E = 64, 512, 50000, 300
H1, H2, D1, D2, C = 256, 512, 128, 64, 1
N_CORES = 8
BPC = B // N_CORES          # batch rows per core = 8
NT = T * BPC                # columns of the transposed activation = 4096
EP = 384                    # E padded to 3 partition chunks (col 300 == 1.0)
KE, K1, K2 = EP // 128, H1 // 128, H2 // 128   # 3, 2, 4
BLK = 16                    # time steps per block = one PSUM bank of xw2
NCOL_BLK = BLK * BPC        # 128 activation columns per block

F16 = mybir.dt.float16
F32 = mybir.dt.float32
I32 = mybir.dt.int32
AF = mybir.ActivationFunctionType

SKEW = BLK + 8              # rnn2 runs SKEW steps behind rnn1

MAX_WAITS = 1  # walrus in this container rejects more sem waits per inst


def _split_excess_waits(nc, max_waits=MAX_WAITS):
    """The container's walrus codegen rejects instructions carrying more than
    a couple of sem waits ("Too many sync wait commands"). Tile freely attaches
    many. Post-process the scheduled BIR: move excess waits onto injected NoOps
    placed immediately before the instruction on the same engine (engines
    process waits in instruction order, so semantics are preserved)."""
    ctr = 0
    for f in nc.m.functions:
        for b in f.blocks:
            new_insts = []
            changed = False
            for inst in b.instructions:
                s = inst.sync_info
                if s is not None and s.on_wait and len(s.on_wait) > max_waits:
                    w = list(s.on_wait)
                    n_extra = len(w) - max_waits
                    for i in range(0, n_extra, max_waits):
                        chunk = w[i : min(i + max_waits, n_extra)]
                        nop = mybir.InstNoOp(
                            name=f"bass_waitsplit_{ctr}",
                            engine=inst.engine,
                            ins=[],
                            outs=[],
                            sync_info=mybir.SyncInfo(on_wait=chunk, on_update=[]),
                        )
                        ctr += 1
                        new_insts.append(nop)
                    s.on_wait = w[n_extra:]
                    changed = True
                new_insts.append(inst)
            if changed:
                b.instructions = new_insts
    return ctr


def _strip_dead_pe_sem_updates(nc):
    """PE semaphore increments serialize at ~26ns each on the engine, which
    caps the matmul issue rate when every MM and LDWEIGHTS carries one. Only
    a small subset of PE-sem values are ever waited on (relu RAW deps etc.).
    Drop the updates nobody waits on and renumber the surviving wait/update
    values (waits are monotone sem>=imm, so rank-renumbering is exact)."""
    all_insts = []
    for f in nc.m.functions:
        for b in f.blocks:
            all_insts.extend(b.instructions)

    upd_engines = {}
    for inst in all_insts:
        s = inst.sync_info
        if s:
            for u in s.on_update:
                upd_engines.setdefault(u.id, set()).add(inst.engine)
    pe_sems = {
        sid for sid, engs in upd_engines.items()
        if engs == {mybir.EngineType.PE}
    }

    needed = {sid: set() for sid in pe_sems}
    for inst in all_insts:
        s = inst.sync_info
        if s:
            for w in s.on_wait:
                if w.id in pe_sems:
                    needed[w.id].add(w.wait_value)

    n_removed = 0
    for sid in pe_sems:
        # walk the PE program order, numbering updates of this sem
        count = 0
        old2new = {}
        keep_vals = needed[sid]
        newc = 0
        pe_insts = [i for i in all_insts if i.engine == mybir.EngineType.PE]
        total = sum(
            1 for i in pe_insts if i.sync_info
            for u in i.sync_info.on_update if u.id == sid
        )
        for inst in pe_insts:
            s = inst.sync_info
            if not s or not s.on_update:
                continue
            kept = []
            for u in s.on_update:
                if u.id != sid:
                    kept.append(u)
                    continue
                count += 1
                # keep waited-on values and the final update (completion)
                if count in keep_vals or count == total:
                    newc += 1
                    old2new[count] = newc
                    kept.append(u)
                else:
                    n_removed += 1
            s.on_update = kept
        # renumber waits: value K maps to the rank of the smallest kept
        # value >= K (same gating instruction)
        kept_sorted = sorted(old2new)
        import bisect
        for inst in all_insts:
            s = inst.sync_info
            if s:
                for w in s.on_wait:
                    if w.id == sid:
                        j = bisect.bisect_left(kept_sorted, w.wait_value)
                        w.wait_value = old2new[kept_sorted[j]]
    return n_removed


def build_nc(t_steps=T):
    """Emit the per-core Bass program. t_steps<T builds a truncated model
    (debug only)."""
    assert t_steps % BLK == 0
    nblk = t_steps // BLK
    nt = t_steps * BPC

    nc = bass.Bass()
    # ---- DRAM I/O (per core) ----
    tok_d = nc.dram_tensor("tokens", [128, nblk], I32, kind="ExternalInput")
    emb_d = nc.dram_tensor("emb", [V, EP], F16, kind="ExternalInput")
    # fp16 weight layouts [p, m, k, col] (see prep_core_inputs)
    w1x_d = nc.dram_tensor("w1x", [128, K1, KE, 128], F16, kind="ExternalInput")
    w1h_d = nc.dram_tensor("w1h", [128, K1, K1, 128], F16, kind="ExternalInput")
    w2x_d = nc.dram_tensor("w2x", [128, K2, K1, 128], F16, kind="ExternalInput")
    w2h_d = nc.dram_tensor("w2h", [128, K2, K2, 128], F16, kind="ExternalInput")
    b2_d = nc.dram_tensor("b2", [128, K2], F32, kind="ExternalInput")
    wd1_d = nc.dram_tensor("wd1", [128, K2, D1], F16, kind="ExternalInput")
    bd1_d = nc.dram_tensor("bd1", [D1, 1], F32, kind="ExternalInput")
    wd2_d = nc.dram_tensor("wd2", [D1, D2], F16, kind="ExternalInput")
    bd2_d = nc.dram_tensor("bd2", [D2, 1], F32, kind="ExternalInput")
    wc_d = nc.dram_tensor("wc", [D2, C], F16, kind="ExternalInput")
    bc_d = nc.dram_tensor("bc", [C, 1], F32, kind="ExternalInput")
    out_d = nc.dram_tensor("out", [C, BPC], F32, kind="ExternalOutput")
    import os
    dbg = os.environ.get("KDBG", "0") == "1"
    if dbg:
        seq1_d = nc.dram_tensor("seq1_dump", [128, K1, nt + BPC], F16,
                                kind="ExternalOutput")
        xw2_d = nc.dram_tensor("xw2_dump", [128, K2, NCOL_BLK], F32,
                               kind="ExternalOutput")

    # The Tile scheduler is greedy-by-priority over READY instructions: it
    # distributes the dependency-free bulk matmuls based on its own latency
    # model, which bunches them and leaves other iterations to pay a ~165ns
    # PE pipe restart at every dependency wait. Chain all PE matmuls in
    # emission order so the static schedule is exactly the emitted
    # interleaving (order-only edges; same-engine, so no semaphore cost).
    pe_prev = [None]

    def pe_mm(**kw):
        inst = nc.tensor.matmul(**kw)
        if pe_prev[0] is not None:
            _add_dep_helper(inst.ins, pe_prev[0].ins, sync=False,
                            reason="pe emission order")
        pe_prev[0] = inst
        return inst

    with tile.TileContext(nc) as tc:
        with (
            tc.tile_pool(name="const", bufs=1) as cpool,
            tc.tile_pool(name="act", bufs=1) as apool,
            tc.tile_pool(name="gath", bufs=4) as gpool,
            tc.tile_pool(name="tmp", bufs=4) as tpool,
            tc.tile_pool(name="ps1", bufs=2, space="PSUM") as ps1,
            tc.tile_pool(name="ps2", bufs=2, space="PSUM") as ps2,
            tc.tile_pool(name="psh", bufs=1, space="PSUM") as psh,
        ):
            # ---- load constants (weights/biases/tokens) ----
            def load(dram, shape, dtype):
                t = cpool.tile(shape, dtype, tag=dram.name)
                nc.sync.dma_start(out=t[:], in_=dram[:])
                return t

            tok_sb = load(tok_d, [128, nblk], I32)
            w1x_sb = load(w1x_d, [128, K1, KE, 128], F16)
            w1h_sb = load(w1h_d, [128, K1, K1, 128], F16)

            # ---- persistent activation buffers (transposed layouts) ----
            xt16 = apool.tile([128, KE, nt], F16, tag="xt16")
            # seq1T doubles as RNN1 state history; col 0:8 is h0=0,
            # step t writes cols 8+8t : 16+8t.
            seq1t = apool.tile([128, K1, nt + BPC], F16, tag="seq1t")
            # RNN2 state ping-pong: cols 0:8 zeros, slots at 8:16, 16:24.
            h2t = apool.tile([128, K2, 3 * BPC], F16, tag="h2t")
            zero16 = apool.tile([128, BLK, K1, BPC], F16, tag="zero16")
            out_sb = apool.tile([C, BPC], F32, tag="out_sb")

            nc.vector.memzero(seq1t[:, :, 0:BPC])
            nc.vector.memzero(h2t[:])
            nc.vector.memset(zero16[:], 0.0)

            # ---- input pipeline: gather + transpose (self-contained on the
            # gpsimd/sync DMA queues) ----
            def input_block(b):
                gt = gpool.tile([128, EP], F16, tag="gt")
                nc.gpsimd.indirect_dma_start(
                    out=gt[:],
                    out_offset=None,
                    in_=emb_d[:],
                    in_offset=bass.IndirectOffsetOnAxis(
                        ap=tok_sb[:, b : b + 1], axis=0
                    ),
                )
                for c in range(KE):
                    nc.sync.dma_start(
                        out=xt16[:, c, b * 128 : (b + 1) * 128],
                        in_=gt[:, c * 128 : (c + 1) * 128],
                        transpose=True,
                    )

            for b in range(min(2, nblk)):
                input_block(b)
            w2x_sb = load(w2x_d, [128, K2, K1, 128], F16)
            w2h_sb = load(w2h_d, [128, K2, K2, 128], F16)
            b2_sb = load(b2_d, [128, K2], F32)
            wd1_sb = load(wd1_d, [128, K2, D1], F16)
            bd1_sb = load(bd1_d, [D1, 1], F32)
            wd2_sb = load(wd2_d, [D1, D2], F16)
            bd2_sb = load(bd2_d, [D2, 1], F32)
            wc_sb = load(wc_d, [D2, C], F16)
            bc_sb = load(bc_d, [C, 1], F32)
            for b in range(2, nblk):
                input_block(b)

            # ---- block-level bulk work ----
            # Each bulk matmul is emitted individually so the main loop can
            # sprinkle them between the recurrent bursts: a dependency-free
            # matmul between two waiting bursts keeps the PE pipe from
            # draining (an isolated restart costs ~165ns vs ~27ns streamed).
            xw1_ps = {}
            xw2_ps = {}

            def xw1_prep(b):
                # allocate + zero the block's PSUM two iterations before the
                # first xw1 filler, on DVE (ScalarE must stay clear for the
                # rnn2 half0 relu; DVE has ~1.4us of slack before first use)
                p = ps1.tile([128, BLK, K1, BPC], F32, tag="p1")
                xw1_ps[b] = p
                nc.vector.memzero(p[:])

            def xw1_mm(b, i):
                # step-major layout: [p, step, m, batch] so each step's
                # region is one contiguous box (no false WAR overlap)
                p = xw1_ps[b]
                m, k = divmod(i, KE)
                sl = slice(b * NCOL_BLK, (b + 1) * NCOL_BLK)
                pe_mm(
                    out=p[:, :, m, :], lhsT=w1x_sb[:, m, k, :],
                    rhs=xt16[:, k, sl], start=False, stop=False,
                    skip_group_check=True,
                )

            def xw1bulk(b):
                xw1_prep(b)
                for i in range(K1 * KE):
                    xw1_mm(b, i)

            def xw2bias(b, m):
                if m == 0:
                    pa = ps2.tile([128, BLK, 2, BPC], F32, tag="p2a")
                    pb = ps2.tile([128, BLK, 2, BPC], F32, tag="p2b")
                    xw2_ps[b] = (pa, pb)
                pt = xw2_ps[b][m // 2]
                nc.scalar.activation(
                    out=pt[:, :, m % 2, :], in_=zero16[:, :, 0, :],
                    func=AF.Identity,
                    bias=b2_sb[:, m : m + 1], scale=1.0,
                )

            def xw2_mm(b, i):
                pa, pb = xw2_ps[b]
                m, k = divmod(i, K1)
                pt = (pa, pb)[m // 2]
                sl = slice(BPC + b * NCOL_BLK, BPC + (b + 1) * NCOL_BLK)
                pe_mm(
                    out=pt[:, :, m % 2, :], lhsT=w2x_sb[:, m, k, :],
                    rhs=seq1t[:, k, sl], start=False, stop=False,
                    skip_group_check=True,
                )

            # ---- recurrent steps ----
            def rnn1_step(t):
                p = xw1_ps[t // BLK]
                ti = t % BLK
                for k in range(K1):
                    for m in range(K1):
                        pe_mm(
                            out=p[:, ti, m, :],
                            lhsT=w1h_sb[:, m, k, :],
                            rhs=seq1t[:, k, t * BPC : (t + 1) * BPC],
                            start=False, stop=(k == K1 - 1),
                            skip_group_check=True,
                        )
                # ScalarE: keeps DVE clear for the critical rnn2 relu0; this
                # relu has ~700ns of slack before its consumer
                nc.scalar.activation(
                    out=seq1t[:, :, (t + 1) * BPC : (t + 2) * BPC],
                    in_=p[:, ti, :, :], func=AF.Relu,
                )

            def rnn2_step(t):
                pts = xw2_ps[t // BLK]
                ti = t % BLK
                src = 0 if t == 0 else BPC + ((t - 1) % 2) * BPC
                dst = BPC + (t % 2) * BPC
                for half in (0, 1):
                    pt = pts[half]
                    # k-interleaved: freshest state chunks consumed last
                    for k in range(K2):
                        for mloc in (0, 1):
                            pe_mm(
                                out=pt[:, ti, mloc, :],
                                lhsT=w2h_sb[:, 2 * half + mloc, k, :],
                                rhs=h2t[:, k, src : src + BPC],
                                start=False, stop=(k == K2 - 1),
                                skip_group_check=True,
                            )
                    # relu0 feeds the very first matmuls of the next burst:
                    # it needs the fast engine (DVE, 173ns vs ScalarE 274ns)
                    # AND an empty queue (rnn1's relu lives on ScalarE so
                    # nothing queues ahead of relu0 here). relu1's consumers
                    # sit 5+ matmuls into the next burst, so ScalarE's extra
                    # 100ns exec is covered.
                    with tc.high_priority(40):
                        if half == 0:
                            nc.vector.tensor_scalar_max(
                                h2t[:, 0:2, dst : dst + BPC],
                                pt[:, ti, :, :], 0.0,
                            )
                        else:
                            nc.scalar.activation(
                                out=h2t[:, 2:4, dst : dst + BPC],
                                in_=pt[:, ti, :, :], func=AF.Relu,
                            )

            # ---- main pipeline: layer-2 runs SKEW steps behind layer-1 ----
            # Per iteration: rnn2 burst (critical chain), one dependency-free
            # bulk matmul (keeps the PE pipe fed while the relu sems land),
            # then the rnn1 burst. Bias preloads spread one-per-iteration so
            # they never queue ahead of the rnn1 relu on ScalarE.
            xw1bulk(0)
            for blk in range(nblk):
                for ti in range(BLK):
                    t = blk * BLK + ti
                    # rnn2 first: its relus are the critical chain
                    t2 = t - SKEW
                    if t2 >= 0:
                        rnn2_step(t2)
                    # filler: xw2 of the previous block (consumed starting at
                    # ti==8 by rnn2), then xw1 of the next block
                    if ti < 8:
                        if blk > 0:
                            xw2_mm(blk - 1, ti)
                    elif ti < 8 + K1 * KE and blk + 1 < nblk:
                        xw1_mm(blk + 1, ti - 8)
                    if ti == 6 and blk + 1 < nblk:
                        xw1_prep(blk + 1)
                    rnn1_step(t)
                    if 10 <= ti < 10 + K2:
                        xw2bias(blk, ti - 10)
            for j, t2 in enumerate(range(t_steps - SKEW, t_steps)):
                rnn2_step(t2)
                if j < K2 * K1:
                    xw2_mm(nblk - 1, j)

            # ---- dense head on the final RNN2 state ----
            t_last = t_steps - 1
            hfin = h2t[:, :, BPC + (t_last % 2) * BPC : 2 * BPC + (t_last % 2) * BPC]

            ps = psh.tile([D1, BPC], F32, tag="h")
            for k in range(K2):
                pe_mm(out=ps[:], lhsT=wd1_sb[:, k, :],
                      rhs=hfin[:, k, :],
                      start=(k == 0), stop=(k == K2 - 1))
            d1 = tpool.tile([D1, BPC], F16, tag="d1")
            nc.scalar.activation(out=d1[:], in_=ps[:], func=AF.Relu,
                                 bias=bd1_sb[:, 0:1], scale=1.0)

            ps = psh.tile([D2, BPC], F32, tag="h")
            pe_mm(out=ps[:], lhsT=wd2_sb[:], rhs=d1[:], start=True,
                  stop=True)
            d2 = tpool.tile([D2, BPC], F16, tag="d2")
            nc.scalar.activation(out=d2[:], in_=ps[:], func=AF.Relu,
                                 bias=bd2_sb[:, 0:1], scale=1.0)

            ps = psh.tile([C, BPC], F32, tag="h")
            pe_mm(out=ps[:], lhsT=wc_sb[:], rhs=d2[:], start=True,
                  stop=True)
            nc.scalar.activation(out=out_sb[:], in_=ps[:], func=AF.Sigmoid,
                                 bias=bc_sb[:, 0:1], scale=1.0)
            nc.sync.dma_start(out=out_d[:], in_=out_sb[:])
            if dbg:
                nc.sync.dma_start(out=seq1_d[:], in_=seq1t[:])
                xw2c = apool.tile([128, K2, NCOL_BLK], F32, tag="xw2c")
                nc.vector.tensor_scalar_add(xw2c[:], xw2_ps[nblk - 1][:], 0.0)
                nc.sync.dma_start(out=xw2_d[:], in_=xw2c[:])

    n_split = _split_excess_waits(nc)
    n_strip = _strip_dead_pe_sem_updates(nc)
    print(f"[kernel] split {n_split} excess-wait NoOps, "
          f"stripped {n_strip} dead PE sem updates")
    return nc


# ---------------------------------------------------------------------------
# Host-side input prep


def prep_core_inputs(inputs, t_steps=T):
    """Returns (shared_weight_map, per_core_token_list)."""
    emb = np.asarray(inputs["emb"], np.float32)
    emb_p = np.zeros((V, EP), np.float16)
    emb_p[:, :E] = emb.astype(np.float16)
    emb_p[:, E] = 1.0  # constant-one feature carries b1 (W1x_pad row E = b1)

    w1x = np.zeros((EP, H1), np.float32)
    w1x[:E] = np.asarray(inputs["W1x"], np.float32)
    w1x[E] = np.asarray(inputs["b1"], np.float32)

    w1h = np.asarray(inputs["W1h"], np.float32)
    w2x = np.asarray(inputs["W2x"], np.float32)
    w2h = np.asarray(inputs["W2h"], np.float32)

    def _pm(w, kc, mc):
        """[K, M] -> [p, m, k, col] fp16."""
        return np.ascontiguousarray(
            w.reshape(kc, 128, mc, 128).transpose(1, 2, 0, 3)
        ).astype(np.float16)

    shared = {
        "emb": emb_p,
        "w1x": _pm(w1x, KE, K1),
        "w1h": _pm(w1h, K1, K1),
        "w2x": _pm(w2x, K1, K2),
        "w2h": _pm(w2h, K2, K2),
        "b2": np.ascontiguousarray(
            np.asarray(inputs["b2"], np.float32).reshape(K2, 128).T
        ),
        "wd1": np.ascontiguousarray(
            np.asarray(inputs["Wd1"], np.float32).reshape(K2, 128, D1)
            .transpose(1, 0, 2)
        ).astype(np.float16),
        "bd1": np.asarray(inputs["bd1"], np.float32).reshape(D1, 1),
        "wd2": np.asarray(inputs["Wd2"], np.float32).astype(np.float16),
        "bd2": np.asarray(inputs["bd2"], np.float32).reshape(D2, 1),
        "wc": np.asarray(inputs["Wc"], np.float32).astype(np.float16),
        "bc": np.asarray(inputs["bc"], np.float32).reshape(C, 1),
    }

    tokens = np.asarray(inputs["tokens"], np.int32)
    per_core_tok = []
    gath_tiles = (t_steps * BPC) // 128
    for c in range(N_CORES):
        cols = tokens[c * BPC : (c + 1) * BPC, :t_steps].T.reshape(-1)  # (t,b)
        per_core_tok.append(
            np.ascontiguousarray(cols.reshape(gath_tiles, 128).T)
        )
    return shared, per_core_tok


_CACHE = {}


def run(inputs, t_steps=T, trace=False):
    key = t_steps
    if key not in _CACHE:
        _CACHE[key] = build_nc(t_steps)
    nc = _CACHE[key]
    shared, per_core_tok = prep_core_inputs(inputs, t_steps)
    in_maps = [dict(shared, tokens=per_core_tok[c]) for c in range(N_CORES)]
    res = run_bass_kernel_spmd(
        nc, in_maps, core_ids=list(range(N_CORES)), trace=trace
    )
    out = np.concatenate(
        [res.results[c]["out"].reshape(BPC, C) for c in range(N_CORES)], axis=0
    )
    return out.astype(np.float32), res


def kernel(**inputs):
    out, _ = run(inputs)
    return out



# revision 25
# speedup vs baseline: 1.0158x; 1.0158x over previous
"""Trainium2 Bass kernel for a 2-layer SimpleRNN classifier (v3).

Model (per reference):
  x = emb[tokens]                               # [B,T,E]
  seq1 = SimpleRNN_relu(x;  W1x, W1h, b1)       # [B,T,H1], return_sequences
  h    = SimpleRNN_relu(seq1; W2x, W2h, b2)[-1] # [B,H2], last step
  h = relu(h@Wd1+bd1); h = relu(h@Wd2+bd2); out = sigmoid(h@Wc+bc)  # [B,1]

Sharding: data-parallel over batch, 8 rows per core on 8 NeuronCores.
Activations transposed on-chip (features on partitions, (time,batch) on
the free dim). Structure (v3, fp16 operands / fp32 PSUM):
  - the bulk input projections (xw) accumulate directly in PSUM
    (16-step blocks, one PSUM bank each, double buffered); the
    recurrent Wh matmuls accumulate on top (start=False) and the
    per-step relu is one fused tensor_scalar max from PSUM (no identity
    matmuls, no PSUM->SBUF xw copies);
  - b1 is folded into the embedding via a constant-one padding column
    (x_pad[...,300]=1, W1x_pad[300,:]=b1); b2 is preloaded into each
    PSUM block by ScalarE, emitted mid-block so it hides behind rnn1;
  - recurrent matmul order is k-interleaved so matmuls consuming the
    freshest state chunks issue as late as possible.
fp8 was measured and rejected: PE stationary load for fp8 has a
~107-132ns/instruction floor on this hw vs ~32ns for fp16, so one fp8
DoubleRow (2 chunks) loses to the two fp16 matmuls it replaces.
"""

import numpy as np

import concourse.bass as bass
import concourse.mybir as mybir
import concourse.tile as tile
from concourse.bass import _add_dep_helper
from concourse.bass_utils import run_bass_kernel_spmd

# ---------------------------------------------------------------------------
# Problem constants (hardcoded per the task contract).
B, T, V, E = 64, 512, 50000, 300
H1, H2, D1, D2, C = 256, 512, 128, 64, 1
N_CORES = 8
BPC = B // N_CORES          # batch rows per core = 8
NT = T * BPC                # columns of the transposed activation = 4096
EP = 384                    # E padded to 3 partition chunks (col 300 == 1.0)
KE, K1, K2 = EP // 128, H1 // 128, H2 // 128   # 3, 2, 4
BLK = 16                    # time steps per block = one PSUM bank of xw2
NCOL_BLK = BLK * BPC        # 128 activation columns per block

F16 = mybir.dt.float16
F32 = mybir.dt.float32
I32 = mybir.dt.int32
AF = mybir.ActivationFunctionType

SKEW = BLK + 8              # rnn2 runs SKEW steps behind rnn1

MAX_WAITS = 1  # walrus in this container rejects more sem waits per inst


def _split_excess_waits(nc, max_waits=MAX_WAITS):
    """The container's walrus codegen rejects instructions carrying more than
    a couple of sem waits ("Too many sync wait commands"). Tile freely attaches
    many. Post-process the scheduled BIR: move excess waits onto injected NoOps
    placed immediately before the instruction on the same engine (engines
    process waits in instruction order, so semantics are preserved)."""
    ctr = 0
    for f in nc.m.functions:
        for b in f.blocks:
            new_insts = []
            changed = False
            for inst in b.instructions:
                s = inst.sync_info
                if s is not None and s.on_wait and len(s.on_wait) > max_waits:
                    w = list(s.on_wait)
                    n_extra = len(w) - max_waits
                    for i in range(0, n_extra, max_waits):
                        chunk = w[i : min(i + max_waits, n_extra)]
                        nop = mybir.InstNoOp(
                            name=f"bass_waitsplit_{ctr}",
                            engine=inst.engine,
                            ins=[],
                            outs=[],
                            sync_info=mybir.SyncInfo(on_wait=chunk, on_update=[]),
                        )
                        ctr += 1
                        new_insts.append(nop)
                    s.on_wait = w[n_extra:]
                    changed = True
                new_insts.append(inst)
            if changed:
                b.instructions = new_insts
    return ctr


def _strip_dead_pe_sem_updates(nc):
    """PE semaphore increments serialize at ~26ns each on the engine, which
    caps the matmul issue rate when every MM and LDWEIGHTS carries one. Only
    a small subset of PE-sem values are ever waited on (relu RAW deps etc.).
    Drop the updates nobody waits on and renumber the surviving wait/update
    values (waits are monotone sem>=imm, so rank-renumbering is exact)."""
    all_insts = []
    for f in nc.m.functions:
        for b in f.blocks:
            all_insts.extend(b.instructions)

    upd_engines = {}
    for inst in all_insts:
        s = inst.sync_info
        if s:
            for u in s.on_update:
                upd_engines.setdefault(u.id, set()).add(inst.engine)
    pe_sems = {
        sid for sid, engs in upd_engines.items()
        if engs == {mybir.EngineType.PE}
    }

    needed = {sid: set() for sid in pe_sems}
    for inst in all_insts:
        s = inst.sync_info
        if s:
            for w in s.on_wait:
                if w.id in pe_sems:
                    needed[w.id].add(w.wait_value)

    n_removed = 0
    for sid in pe_sems:
        # walk the PE program order, numbering updates of this sem
        count = 0
        old2new = {}
        keep_vals = needed[sid]
        newc = 0
        pe_insts = [i for i in all_insts if i.engine == mybir.EngineType.PE]
        total = sum(
            1 for i in pe_insts if i.sync_info
            for u in i.sync_info.on_update if u.id == sid
        )
        for inst in pe_insts:
            s = inst.sync_info
            if not s or not s.on_update:
                continue
            kept = []
            for u in s.on_update:
                if u.id != sid:
                    kept.append(u)
                    continue
                count += 1
                # keep waited-on values and the final update (completion)
                if count in keep_vals or count == total:
                    newc += 1
                    old2new[count] = newc
                    kept.append(u)
                else:
                    n_removed += 1
            s.on_update = kept
        # renumber waits: value K maps to the rank of the smallest kept
        # value >= K (same gating instruction)
        kept_sorted = sorted(old2new)
        import bisect
        for inst in all_insts:
            s = inst.sync_info
            if s:
                for w in s.on_wait:
                    if w.id == sid:
                        j = bisect.bisect_left(kept_sorted, w.wait_value)
                        w.wait_value = old2new[kept_sorted[j]]
    return n_removed


def build_nc(t_steps=T):
    """Emit the per-core Bass program. t_steps<T builds a truncated model
    (debug only)."""
    assert t_steps % BLK == 0
    nblk = t_steps // BLK
    nt = t_steps * BPC

    nc = bass.Bass()
    # ---- DRAM I/O (per core) ----
    tok_d = nc.dram_tensor("tokens", [128, nblk], I32, kind="ExternalInput")
    emb_d = nc.dram_tensor("emb", [V, EP], F16, kind="ExternalInput")
    # fp16 weight layouts [p, m, k, col] (see prep_core_inputs)
    w1x_d = nc.dram_tensor("w1x", [128, K1, KE, 128], F16, kind="ExternalInput")
    w1h_d = nc.dram_tensor("w1h", [128, K1, K1, 128], F16, kind="ExternalInput")
    w2x_d = nc.dram_tensor("w2x", [128, K2, K1, 128], F16, kind="ExternalInput")
    w2h_d = nc.dram_tensor("w2h", [128, K2, K2, 128], F16, kind="ExternalInput")
    b2_d = nc.dram_tensor("b2", [128, K2], F32, kind="ExternalInput")
    wd1_d = nc.dram_tensor("wd1", [128, K2, D1], F16, kind="ExternalInput")
    bd1_d = nc.dram_tensor("bd1", [D1, 1], F32, kind="ExternalInput")
    wd2_d = nc.dram_tensor("wd2", [D1, D2], F16, kind="ExternalInput")
    bd2_d = nc.dram_tensor("bd2", [D2, 1], F32, kind="ExternalInput")
    wc_d = nc.dram_tensor("wc", [D2, C], F16, kind="ExternalInput")
    bc_d = nc.dram_tensor("bc", [C, 1], F32, kind="ExternalInput")
    out_d = nc.dram_tensor("out", [C, BPC], F32, kind="ExternalOutput")
    import os
    dbg = os.environ.get("KDBG", "0") == "1"
    if dbg:
        seq1_d = nc.dram_tensor("seq1_dump", [128, K1, nt + BPC], F16,
                                kind="ExternalOutput")
        xw2_d = nc.dram_tensor("xw2_dump", [128, K2, NCOL_BLK], F32,
                               kind="ExternalOutput")

    # The Tile scheduler is greedy-by-priority over READY instructions: it
    # distributes the dependency-free bulk matmuls based on its own latency
    # model, which bunches them and leaves other iterations to pay a ~165ns
    # PE pipe restart at every dependency wait. Chain all PE matmuls in
    # emission order so the static schedule is exactly the emitted
    # interleaving (order-only edges; same-engine, so no semaphore cost).
    pe_prev = [None]

    def pe_mm(**kw):
        inst = nc.tensor.matmul(**kw)
        if pe_prev[0] is not None:
            _add_dep_helper(inst.ins, pe_prev[0].ins, sync=False,
                            reason="pe emission order")
        pe_prev[0] = inst
        return inst

    with tile.TileContext(nc) as tc:
        with (
            tc.tile_pool(name="const", bufs=1) as cpool,
            tc.tile_pool(name="act", bufs=1) as apool,
            tc.tile_pool(name="gath", bufs=4) as gpool,
            tc.tile_pool(name="tmp", bufs=4) as tpool,
            tc.tile_pool(name="ps1", bufs=2, space="PSUM") as ps1,
            tc.tile_pool(name="ps2", bufs=2, space="PSUM") as ps2,
            tc.tile_pool(name="psh", bufs=1, space="PSUM") as psh,
        ):
            # ---- load constants (weights/biases/tokens) ----
            def load(dram, shape, dtype):
                t = cpool.tile(shape, dtype, tag=dram.name)
                nc.sync.dma_start(out=t[:], in_=dram[:])
                return t

            tok_sb = load(tok_d, [128, nblk], I32)
            w1x_sb = load(w1x_d, [128, K1, KE, 128], F16)
            w1h_sb = load(w1h_d, [128, K1, K1, 128], F16)

            # ---- persistent activation buffers (transposed layouts) ----
            xt16 = apool.tile([128, KE, nt], F16, tag="xt16")
            # seq1T doubles as RNN1 state history; col 0:8 is h0=0,
            # step t writes cols 8+8t : 16+8t.
            seq1t = apool.tile([128, K1, nt + BPC], F16, tag="seq1t")
            # RNN2 state ping-pong: cols 0:8 zeros, slots at 8:16, 16:24.
            h2t = apool.tile([128, K2, 3 * BPC], F16, tag="h2t")
            zero16 = apool.tile([128, BLK, K1, BPC], F16, tag="zero16")
            out_sb = apool.tile([C, BPC], F32, tag="out_sb")

            nc.vector.memzero(seq1t[:, :, 0:BPC])
            nc.vector.memzero(h2t[:])
            nc.vector.memset(zero16[:], 0.0)

            # ---- input pipeline: gather + transpose (self-contained on the
            # gpsimd/sync DMA queues) ----
            def input_block(b):
                gt = gpool.tile([128, EP], F16, tag="gt")
                nc.gpsimd.indirect_dma_start(
                    out=gt[:],
                    out_offset=None,
                    in_=emb_d[:],
                    in_offset=bass.IndirectOffsetOnAxis(
                        ap=tok_sb[:, b : b + 1], axis=0
                    ),
                )
                for c in range(KE):
                    nc.sync.dma_start(
                        out=xt16[:, c, b * 128 : (b + 1) * 128],
                        in_=gt[:, c * 128 : (c + 1) * 128],
                        transpose=True,
                    )

            for b in range(min(2, nblk)):
                input_block(b)
            w2x_sb = load(w2x_d, [128, K2, K1, 128], F16)
            w2h_sb = load(w2h_d, [128, K2, K2, 128], F16)
            b2_sb = load(b2_d, [128, K2], F32)
            wd1_sb = load(wd1_d, [128, K2, D1], F16)
            bd1_sb = load(bd1_d, [D1, 1], F32)
            wd2_sb = load(wd2_d, [D1, D2], F16)
            bd2_sb = load(bd2_d, [D2, 1], F32)
            wc_sb = load(wc_d, [D2, C], F16)
            bc_sb = load(bc_d, [C, 1], F32)
            for b in range(2, nblk):
                input_block(b)

            # ---- block-level bulk work ----
            # Each bulk matmul is emitted individually so the main loop can
            # sprinkle them between the recurrent bursts: a dependency-free
            # matmul between two waiting bursts keeps the PE pipe from
            # draining (an isolated restart costs ~165ns vs ~27ns streamed).
            xw1_ps = {}
            xw2_ps = {}

            def xw1_prep(b):
                # allocate + zero the block's PSUM two iterations before the
                # first xw1 filler, on DVE (ScalarE must stay clear for the
                # rnn2 half0 relu; DVE has ~1.4us of slack before first use)
                p = ps1.tile([128, BLK, K1, BPC], F32, tag="p1")
                xw1_ps[b] = p
                nc.vector.memzero(p[:])

            def xw1_mm(b, i):
                # step-major layout: [p, step, m, batch] so each step's
                # region is one contiguous box (no false WAR overlap)
                p = xw1_ps[b]
                m, k = divmod(i, KE)
                sl = slice(b * NCOL_BLK, (b + 1) * NCOL_BLK)
                pe_mm(
                    out=p[:, :, m, :], lhsT=w1x_sb[:, m, k, :],
                    rhs=xt16[:, k, sl], start=False, stop=False,
                    skip_group_check=True,
                )

            def xw1bulk(b):
                xw1_prep(b)
                for i in range(K1 * KE):
                    xw1_mm(b, i)

            def xw2bias(b, m):
                if m == 0:
                    pa = ps2.tile([128, BLK, 2, BPC], F32, tag="p2a")
                    pb = ps2.tile([128, BLK, 2, BPC], F32, tag="p2b")
                    xw2_ps[b] = (pa, pb)
                pt = xw2_ps[b][m // 2]
                nc.scalar.activation(
                    out=pt[:, :, m % 2, :], in_=zero16[:, :, 0, :],
                    func=AF.Identity,
                    bias=b2_sb[:, m : m + 1], scale=1.0,
                )

            def xw2_mm(b, i):
                pa, pb = xw2_ps[b]
                m, k = divmod(i, K1)
                pt = (pa, pb)[m // 2]
                sl = slice(BPC + b * NCOL_BLK, BPC + (b + 1) * NCOL_BLK)
                pe_mm(
                    out=pt[:, :, m % 2, :], lhsT=w2x_sb[:, m, k, :],
                    rhs=seq1t[:, k, sl], start=False, stop=False,
                    skip_group_check=True,
                )

            # ---- recurrent steps ----
            def rnn1_step(t):
                p = xw1_ps[t // BLK]
                ti = t % BLK
                for k in range(K1):
                    for m in range(K1):
                        pe_mm(
                            out=p[:, ti, m, :],
                            lhsT=w1h_sb[:, m, k, :],
                            rhs=seq1t[:, k, t * BPC : (t + 1) * BPC],
                            start=False, stop=(k == K1 - 1),
                            skip_group_check=True,
                        )
                # DVE: ScalarE carries the rnn2 half0 relu; this one has
                # ~700ns of slack before its consumer
                nc.vector.tensor_scalar_max(
                    seq1t[:, :, (t + 1) * BPC : (t + 2) * BPC],
                    p[:, ti, :, :], 0.0,
                )

            def rnn2_step(t):
                pts = xw2_ps[t // BLK]
                ti = t % BLK
                src = 0 if t == 0 else BPC + ((t - 1) % 2) * BPC
                dst = BPC + (t % 2) * BPC
                for half in (0, 1):
                    pt = pts[half]
                    # k-interleaved: freshest state chunks consumed last
                    for k in range(K2):
                        for mloc in (0, 1):
                            pe_mm(
                                out=pt[:, ti, mloc, :],
                                lhsT=w2h_sb[:, 2 * half + mloc, k, :],
                                rhs=h2t[:, k, src : src + BPC],
                                start=False, stop=(k == K2 - 1),
                                skip_group_check=True,
                            )
                    # split the two half-relus across engines: one engine
                    # serializes them and the late one stalls the next
                    # burst. relu0 is ready early (half0 stop) -> DVE;
                    # relu1 must start the instant half1 stops -> ScalarE
                    # (idle at that point in the iteration).
                    with tc.high_priority(40):
                        if half == 0:
                            nc.scalar.activation(
                                out=h2t[:, 0:2, dst : dst + BPC],
                                in_=pt[:, ti, :, :], func=AF.Relu,
                            )
                        else:
                            nc.vector.tensor_scalar_max(
                                h2t[:, 2:4, dst : dst + BPC],
                                pt[:, ti, :, :], 0.0,
                            )

            # ---- main pipeline: layer-2 runs SKEW steps behind layer-1 ----
            # Per iteration: rnn2 burst (critical chain), one dependency-free
            # bulk matmul (keeps the PE pipe fed while the relu sems land),
            # then the rnn1 burst. Bias preloads spread one-per-iteration so
            # they never queue ahead of the rnn1 relu on ScalarE.
            xw1bulk(0)
            for blk in range(nblk):
                for ti in range(BLK):
                    t = blk * BLK + ti
                    # rnn2 first: its relus are the critical chain
                    t2 = t - SKEW
                    if t2 >= 0:
                        rnn2_step(t2)
                    # filler: xw2 of the previous block (consumed starting at
                    # ti==8 by rnn2), then xw1 of the next block
                    if ti < 8:
                        if blk > 0:
                            xw2_mm(blk - 1, ti)
                    elif ti < 8 + K1 * KE and blk + 1 < nblk:
                        xw1_mm(blk + 1, ti - 8)
                    if ti == 6 and blk + 1 < nblk:
                        xw1_prep(blk + 1)
                    rnn1_step(t)
                    if 10 <= ti < 10 + K2:
                        xw2bias(blk, ti - 10)
            for j, t2 in enumerate(range(t_steps - SKEW, t_steps)):
                rnn2_step(t2)
                if j < K2 * K1:
                    xw2_mm(nblk - 1, j)

            # ---- dense head on the final RNN2 state ----
            t_last = t_steps - 1
            hfin = h2t[:, :, BPC + (t_last % 2) * BPC : 2 * BPC + (t_last % 2) * BPC]

            ps = psh.tile([D1, BPC], F32, tag="h")
            for k in range(K2):
                pe_mm(out=ps[:], lhsT=wd1_sb[:, k, :],
                      rhs=hfin[:, k, :],
                      start=(k == 0), stop=(k == K2 - 1))
            d1 = tpool.tile([D1, BPC], F16, tag="d1")
            nc.scalar.activation(out=d1[:], in_=ps[:], func=AF.Relu,
                                 bias=bd1_sb[:, 0:1], scale=1.0)

            ps = psh.tile([D2, BPC], F32, tag="h")
            pe_mm(out=ps[:], lhsT=wd2_sb[:], rhs=d1[:], start=True,
                  stop=True)
            d2 = tpool.tile([D2, BPC], F16, tag="d2")
            nc.scalar.activation(out=d2[:], in_=ps[:], func=AF.Relu,
                                 bias=bd2_sb[:, 0:1], scale=1.0)

            ps = psh.tile([C, BPC], F32, tag="h")
            pe_mm(out=ps[:], lhsT=wc_sb[:], rhs=d2[:], start=True,
                  stop=True)
            nc.scalar.activation(out=out_sb[:], in_=ps[:], func=AF.Sigmoid,
                                 bias=bc_sb[:, 0:1], scale=1.0)
            nc.sync.dma_start(out=out_d[:], in_=out_sb[:])
            if dbg:
                nc.sync.dma_start(out=seq1_d[:], in_=seq1t[:])
                xw2c = apool.tile([128, K2, NCOL_BLK], F32, tag="xw2c")
                nc.vector.tensor_scalar_add(xw2c[:], xw2_ps[nblk - 1][:], 0.0)
                nc.sync.dma_start(out=xw2_d[:], in_=xw2c[:])

    n_split = _split_excess_waits(nc)
    n_strip = _strip_dead_pe_sem_updates(nc)
    print(f"[kernel] split {n_split} excess-wait NoOps, "
          f"stripped {n_strip} dead PE sem updates")
    return nc


# ---------------------------------------------------------------------------
# Host-side input prep


def prep_core_inputs(inputs, t_steps=T):
    """Returns (shared_weight_map, per_core_token_list)."""
    emb = np.asarray(inputs["emb"], np.float32)
    emb_p = np.zeros((V, EP), np.float16)
    emb_p[:, :E] = emb.astype(np.float16)
    emb_p[:, E] = 1.0  # constant-one feature carries b1 (W1x_pad row E = b1)

    w1x = np.zeros((EP, H1), np.float32)
    w1x[:E] = np.asarray(inputs["W1x"], np.float32)
    w1x[E] = np.asarray(inputs["b1"], np.float32)

    w1h = np.asarray(inputs["W1h"], np.float32)
    w2x = np.asarray(inputs["W2x"], np.float32)
    w2h = np.asarray(inputs["W2h"], np.float32)

    def _pm(w, kc, mc):
        """[K, M] -> [p, m, k, col] fp16."""
        return np.ascontiguousarray(
            w.reshape(kc, 128, mc, 128).transpose(1, 2, 0, 3)
        ).astype(np.float16)

    shared = {
        "emb": emb_p,
        "w1x": _pm(w1x, KE, K1),
        "w1h": _pm(w1h, K1, K1),
        "w2x": _pm(w2x, K1, K2),
        "w2h": _pm(w2h, K2, K2),
        "b2": np.ascontiguousarray(
            np.asarray(inputs["b2"], np.float32).reshape(K2, 128).T
        ),
        "wd1": np.ascontiguousarray(
            np.asarray(inputs["Wd1"], np.float32).reshape(K2, 128, D1)
            .transpose(1, 0, 2)
        ).astype(np.float16),
        "bd1": np.asarray(inputs["bd1"], np.float32).reshape(D1, 1),
        "wd2": np.asarray(inputs["Wd2"], np.float32).astype(np.float16),
        "bd2": np.asarray(inputs["bd2"], np.float32).reshape(D2, 1),
        "wc": np.asarray(inputs["Wc"], np.float32).astype(np.float16),
        "bc": np.asarray(inputs["bc"], np.float32).reshape(C, 1),
    }

    tokens = np.asarray(inputs["tokens"], np.int32)
    per_core_tok = []
    gath_tiles = (t_steps * BPC) // 128
    for c in range(N_CORES):
        cols = tokens[c * BPC : (c + 1) * BPC, :t_steps].T.reshape(-1)  # (t,b)
        per_core_tok.append(
            np.ascontiguousarray(cols.reshape(gath_tiles, 128).T)
        )
    return shared, per_core_tok


_CACHE = {}


def run(inputs, t_steps=T, trace=False):
    key = t_steps
    if key not in _CACHE:
        _CACHE[key] = build_nc(t_steps)
    nc = _CACHE[key]
    shared, per_core_tok = prep_core_inputs(inputs, t_steps)
    in_maps = [dict(shared, tokens=per_core_tok[c]) for c in range(N_CORES)]
    res = run_bass_kernel_spmd(
        nc, in_maps, core_ids=list(range(N_CORES)), trace=trace
    )
    out = np.concatenate(
        [res.results[c]["out"].reshape(BPC, C) for c in range(N_CORES)], axis=0
    )
    return out.astype(np.float32), res


def kernel(**inputs):
    out, _ = run(inputs)
    return out

